# revision 3
# baseline (speedup 1.0000x reference)
"""Trainium2 Bass kernel for int8 GEMM + fp32 bias (linear_a8_w8_bfp32_ofp32).

Computes out = (x_int8 @ weight_int8.T).astype(f32) + bias  for
x [8192, 4096] int8, weight [4096, 4096] int8, bias [4096] f32.

Strategy: column-parallel tensor parallelism over 8 NeuronCores — each core
gets all of x (replicated) and a 512-column slice of weight/bias, and
computes its [8192, 512] output slice.

The PE array has no int8 matmul mode (TRN2/cayman dropped UINT8), but
int8 values are exactly representable in bf16, bf16 x bf16 products
(<= 127*127) are exact, and PSUM accumulates in fp32 where every partial
sum of this data stays far below 2^24 — so a bf16 matmul reproduces the
int32-accumulated reference bit-exactly. fp8 can't beat this: an exact
int8 GEMM needs a >=3x nibble decomposition but DoubleRow only buys
~1.5-1.8x, so bf16 N=512 streaming (215.6 ns/MM) is the PE floor:
2048 MMs = 441.5 us/core.

v2 startup (vs the warmup-matmul baseline at 463.6us): no warmup — the
PE starts cold on real data as early as possible and warms while doing
useful work, which also removes the HAM re-throttle the baseline hit
when its warmup ran dry.  The first w k-tiles ride the otherwise-idle
sync+scalar HWDGE queues (parallel descriptor-gen with the gpsimd SWDGE
ring), w is staged as raw int8 and cast per-k-tile into 32 separate
[128,512] bf16 tiles, alternating DVE (even k) / scalar (odd k) so cast
throughput stays ahead of MM consumption.  x rides gpsimd casting DMAs
(int8->bf16 in the DMA) in 3 startup chunks for m-tile 0, then whole
tiles.  The last m-tile is split into two 256-wide PSUM chains so the
first half's bias-add + store hide behind the second half's matmuls.
"""

import numpy as np

import concourse.mybir as mybir
import concourse.tile as tile
from concourse import bacc
from concourse.bass_utils import run_bass_kernel_spmd

P = 128
N_CORES = 8

# Set by a test harness to capture timing/trace info; harmless defaults.
TRACE = False
TRACE_KWARGS = {}
LAST_RESULT = None


def build_program(MT, KT, NLOC, x_bufs=4, o_bufs=3, psum_bufs=4):
    """Bass/Tile program for one core: out[MT*128, NLOC] = xT.T @ wT + bias.

    DRAM layouts (host pre-arranged, all contiguous per SBUF partition):
      x_tiles   [MT, P, KT, P]  int8   x_tiles[mt, ki, kt, mi] = x[mt*P+mi, kt*P+ki]
      w_tiles   [P, KT, NLOC]   int8   w_tiles[ki, kt, n] = weight[n, kt*P+ki]
      bias_bcast[P, NLOC]       f32    bias replicated across partitions
      out_tiles [MT, P, NLOC]   f32    out_tiles[mt, mi, n] = out[mt*P+mi, n]
    """
    nc = bacc.Bacc()
    x_d = nc.declare_dram_parameter(
        "x_tiles", [MT, P, KT, P], mybir.dt.int8, isOutput=False
    )
    w_d = nc.declare_dram_parameter(
        "w_tiles", [P, KT, NLOC], mybir.dt.int8, isOutput=False
    )
    b_d = nc.declare_dram_parameter(
        "bias_bcast", [P, NLOC], mybir.dt.float32, isOutput=False
    )
    o_d = nc.declare_dram_parameter(
        "out_tiles", [MT, P, NLOC], mybir.dt.float32, isOutput=True
    )

    # w staging chunks (raw int8): (queue, k_start, k_end)
    W_CHUNKS = [
        ("sync", 0, 2),
        ("sync", 2, 4),
        ("scalar", 4, 8),
        ("gpsimd", 8, 20),
        ("gpsimd", 20, 32),
    ]
    # x m-tile 0 startup chunks (casting DMA): (k_start, k_end)
    X0_CHUNKS = [(0, 4), (4, 16), (16, 32)]
    NH = NLOC // 2

    with tile.TileContext(nc) as tc:
        with (
            tc.tile_pool(name="wkpool", bufs=1) as wkpool,
            tc.tile_pool(name="wqpool", bufs=1) as wqpool,
            tc.tile_pool(name="cpool", bufs=1) as cpool,
            tc.tile_pool(name="x0pool", bufs=1) as x0pool,
            tc.tile_pool(name="xpool", bufs=x_bufs) as xpool,
            tc.tile_pool(name="opool", bufs=o_bufs) as opool,
            tc.tile_pool(name="otail", bufs=2) as otail,
            tc.tile_pool(name="psum", bufs=psum_bufs, space="PSUM") as psum_pool,
            tc.tile_pool(name="pst", bufs=2, space="PSUM") as pst_pool,
        ):
            # --- startup DMA emission -------------------------------------
            # sync(SP) + scalar(Act) HWDGE queues carry the first w k-tiles
            # (descriptor-gen in parallel with the gpsimd ring); gpsimd
            # (SWDGE, the only casting-capable queue) carries x plus the
            # later, bigger w chunks.  Emission order per queue = execution
            # order, so the most-urgent chunks are emitted first.
            wq = []
            for qi, (eng, k0, k1) in enumerate(W_CHUNKS):
                wq_t = wqpool.tile([P, k1 - k0, NLOC], mybir.dt.int8, tag=f"wq{qi}")
                wq.append(wq_t)
                if eng != "gpsimd":
                    getattr(nc, eng).dma_start(out=wq_t[:], in_=w_d[:, k0:k1, :])
            b_sb = cpool.tile([P, NLOC], mybir.dt.float32)
            nc.sync.dma_start(out=b_sb[:], in_=b_d[:])

            x0_sb = []
            gp_emits = []  # (k0, k1) w chunks interleaved with x0 chunks
            for ci, (k0, k1) in enumerate(X0_CHUNKS):
                x_c = x0pool.tile([P, k1 - k0, P], mybir.dt.bfloat16, tag=f"x0c{ci}")
                nc.gpsimd.dma_start(out=x_c[:], in_=x_d[0, :, k0:k1, :])
                x0_sb.append(x_c)
                # after x0c1 emit w k8-19, after x0c2 emit w k20-31
                if ci >= 1:
                    qi = ci + 2
                    _, wk0, wk1 = W_CHUNKS[qi]
                    nc.gpsimd.dma_start(out=wq[qi][:], in_=w_d[:, wk0:wk1, :])
            x1_sb = xpool.tile([P, KT, P], mybir.dt.bfloat16)
            nc.gpsimd.dma_start(out=x1_sb[:], in_=x_d[1])

            # --- w casts: per k-tile, DVE (even k) / scalar (odd k) -------
            def w_stage_slice(k):
                for qi, (_, k0, k1) in enumerate(W_CHUNKS):
                    if k0 <= k < k1:
                        return wq[qi][:, k - k0, :]
                raise AssertionError(k)

            wk = []
            for k in range(KT):
                w_t = wkpool.tile([P, NLOC], mybir.dt.bfloat16, tag=f"wk{k}")
                wk.append(w_t)
            for k in range(KT):
                if k % 2 == 0:
                    nc.vector.tensor_copy(wk[k][:], w_stage_slice(k))
                else:
                    nc.scalar.copy(wk[k][:], w_stage_slice(k))

            # --- main m-tile loop -----------------------------------------
            def x_slice(mt, x_sb, kt):
                if mt == 0:
                    for ci, (k0, k1) in enumerate(X0_CHUNKS):
                        if k0 <= kt < k1:
                            return x0_sb[ci][:, kt - k0, :]
                    raise AssertionError(kt)
                return x_sb[:, kt, :]

            for mt in range(MT):
                if mt == 0:
                    x_sb = None
                elif mt == 1:
                    x_sb = x1_sb
                else:
                    x_sb = xpool.tile([P, KT, P], mybir.dt.bfloat16)
                    nc.gpsimd.dma_start(out=x_sb[:], in_=x_d[mt])
                if mt < MT - 1:
                    ps = psum_pool.tile([P, NLOC], mybir.dt.float32)
                    for kt in range(KT):
                        nc.tensor.matmul(
                            ps[:],
                            x_slice(mt, x_sb, kt),
                            wk[kt][:],
                            start=(kt == 0),
                            stop=(kt == KT - 1),
                        )
                    o_sb = opool.tile([P, NLOC], mybir.dt.float32)
                    nc.vector.tensor_add(o_sb[:], ps[:], b_sb[:])
                    nc.sync.dma_start(out=o_d[mt], in_=o_sb[:])
                else:
                    # last m-tile: two 256-wide chains so the first half's
                    # epilogue hides behind the second half's matmuls, and
                    # the final exposed tail is only a half-width epilogue.
                    for h in range(2):
                        ph = pst_pool.tile([P, NH], mybir.dt.float32, tag=f"pst{h}")
                        for kt in range(KT):
                            nc.tensor.matmul(
                                ph[:],
                                x_slice(mt, x_sb, kt),
                                wk[kt][:, h * NH : (h + 1) * NH],
                                start=(kt == 0),
                                stop=(kt == KT - 1),
                            )
                        o_h = otail.tile([P, NH], mybir.dt.float32, tag=f"ot{h}")
                        nc.vector.tensor_add(
                            o_h[:], ph[:], b_sb[:, h * NH : (h + 1) * NH]
                        )
                        # parallel desc-gen: half 0 on sync, half 1 on scalar
                        eng = nc.sync if h == 0 else nc.scalar
                        eng.dma_start(
                            out=o_d[mt, :, h * NH : (h + 1) * NH], in_=o_h[:]
                        )
    nc.compile()
    return nc


def run(x, weight, fake_bias):
    global LAST_RESULT
    M, K = x.shape
    N = weight.shape[0]
    assert M % P == 0 and K % P == 0 and N % (N_CORES * P) == 0
    MT, KT, NLOC = M // P, K // P, N // N_CORES

    xb = np.asarray(x).astype(np.int8)
    x_tiles = np.ascontiguousarray(xb.reshape(MT, P, KT, P).transpose(0, 3, 2, 1))
    wb = np.asarray(weight).astype(np.int8)
    bias = np.asarray(fake_bias).astype(np.float32)

    in_maps = []
    for c in range(N_CORES):
        w_loc = wb[c * NLOC : (c + 1) * NLOC, :]  # [NLOC, K]
        w_tiles = np.ascontiguousarray(
            w_loc.T.reshape(KT, P, NLOC).transpose(1, 0, 2)
        )
        b_loc = np.ascontiguousarray(
            np.broadcast_to(bias[None, c * NLOC : (c + 1) * NLOC], (P, NLOC))
        )
        in_maps.append(
            {"x_tiles": x_tiles, "w_tiles": w_tiles, "bias_bcast": b_loc}
        )

    nc = build_program(MT, KT, NLOC)
    res = run_bass_kernel_spmd(
        nc, in_maps, list(range(N_CORES)), trace=TRACE, **TRACE_KWARGS
    )
    LAST_RESULT = res

    outs = [r["out_tiles"].reshape(M, NLOC) for r in res.results]
    return np.concatenate(outs, axis=1).astype(np.float32)


def kernel(x, weight, fake_bias):
    return run(x, weight, fake_bias)
